# revision 47
# baseline (speedup 1.0000x reference)
"""Trainium2 Bass kernel for the ChiSq (histogram_binning) problem.

Per core (128 of 1024 rows, pure data parallel):
  FFT-16384 of template & strain via 2-stage radix-128 matmul FFT with the
  data as the stationary operand (no transposes), Hermitian-halved twiddle
  (k1 <= 64 only; the upper half of each 128-block is reconstructed inside
  stage-3 via a conjugated E-matrix "set B" whose outputs are written
  reversed), fp32r stage-1 and fp16 moving operands for full-rate PE.
  Binning is hierarchical: per-128 block sums -> block-level masked sums on
  [128, 65] arrays, plus one indirect-DMA gather per threshold of the single
  straddling block (HW DGE streams consecutive rows from one offset per
  partition) for the exact fine correction. This avoids both the row-major
  unflatten DMAs and the 15 full-array masked passes of the naive approach.
  Elementwise work is fp16 (2x DVE), split across DVE/GpSimd; PSUM->fp16
  converts (with the CSCALE fold) run on the Activation engine.
"""
import numpy as np
from contextlib import ExitStack

import concourse.bass as bass
import concourse.tile as tile
from concourse import bacc, mybir
from concourse.bass_utils import run_bass_kernel_spmd

F32 = mybir.dt.float32
F32R = mybir.dt.float32r
BF16 = mybir.dt.bfloat16
F16 = mybir.dt.float16
I32 = mybir.dt.int32

SAMPLE_RATE = 2048.0
FFTLENGTH = 8.0
NUM_BINS = 16
N = int(FFTLENGTH * SAMPLE_RATE)       # 16384
NF = N // 2 + 1                        # 8193
DF = 1.0 / FFTLENGTH
CSCALE = 4.0 * DF / (SAMPLE_RATE ** 2)

NCORES = 8
ROWS = 128          # rows per core
GROUPS = 8
GR = 16             # rows per group (DMA granularity)
R = 8               # rows per compute chunk
CPG = GR // R       # chunks per group
NB = 65             # 64 full blocks + tail block (k = 8192)
PITCH = 66          # DRAM row pitch in 128-blocks (alignment pad)
NM = 16             # thresholds m = 0..15 (t_0 = 0)
SSC = float(np.sqrt(CSCALE))   # folded into the stage-3 PSUM->fp16 converts


def _make_consts():
    s = np.sqrt(np.float32(CSCALE))
    n1 = np.arange(128)
    k1 = np.arange(128)
    ang1 = 2 * np.pi * np.outer(n1, k1) / 128.0
    CS = np.concatenate([np.cos(ang1), np.sin(ang1)], axis=1).astype(np.float32)

    n2 = np.arange(128)
    j = np.arange(65)
    angw = 2 * np.pi * np.outer(n2, j) / float(N)
    twr = np.cos(angw)
    twi = -np.sin(angw)
    twr_rep = np.tile(twr, (1, R))    # [128, 260] (r, j) blocks
    twi_rep = np.tile(twi, (1, R))

    k2 = np.arange(65)
    angA = 2 * np.pi * np.outer(n2, k2) / 128.0
    erA = np.cos(angA)
    eipA = np.sin(angA)
    einA = -eipA
    k2b = np.arange(64)
    angB = 2 * np.pi * np.outer(n2, k2b + 1) / 128.0
    ebr = np.cos(angB)
    ebi = np.sin(angB)
    ebin = -ebi
    cbf = np.concatenate(
        [twr_rep, twi_rep, erA, eipA, einA, ebr, ebi, ebin], axis=1)  # [128, 1427]

    ident = np.eye(128, dtype=np.float32)
    mfrac = np.tile((np.arange(16, dtype=np.float32) / 16.0)[None, :], (128, 1))
    row66 = (np.arange(128, dtype=np.float32) * PITCH)[:, None]
    ones65 = np.ones((128, 65), dtype=np.float32)
    cf = np.concatenate([ident, mfrac, row66, ones65], axis=1)  # [128, 210]

    return dict(
        cs=np.ascontiguousarray(CS, np.float32),
        cbf=np.ascontiguousarray(cbf).astype(np.float16),
        cf=np.ascontiguousarray(cf, np.float32),
    )


def _build_program():
    nc = bacc.Bacc("TRN2", target_bir_lowering=False, debug=False,
                   enable_asserts=False, num_devices=NCORES)
    t_in = nc.dram_tensor("t_in", [ROWS, N], F32R, kind="ExternalInput").ap()
    s_in = nc.dram_tensor("s_in", [ROWS, N], F32R, kind="ExternalInput").ap()
    cs_d = nc.dram_tensor("cs", [128, 256], F32R, kind="ExternalInput").ap()
    cbf_d = nc.dram_tensor("cbf", [128, 1427], F16, kind="ExternalInput").ap()
    cf_d = nc.dram_tensor("cf", [128, 210], F32, kind="ExternalInput").ap()
    # interleaved staging table: [..., 0:128] = ph block, [..., 128:256] = cr
    tab_d = nc.dram_tensor("tab_d", [ROWS, PITCH, 256], F16, kind="Internal").ap()
    out = nc.dram_tensor("chisq_out", [ROWS, 1], F32, kind="ExternalOutput").ap()

    AL = mybir.AluOpType
    AX = mybir.AxisListType

    with tile.TileContext(nc, trace_sim=False) as tc, ExitStack() as ctx:
        cpool = ctx.enter_context(tc.tile_pool(name="consts", bufs=1))
        inp = ctx.enter_context(tc.tile_pool(name="inp", bufs=3))
        ybfp = ctx.enter_context(tc.tile_pool(name="ybfp", bufs=3))
        zp = ctx.enter_context(tc.tile_pool(name="zp", bufs=3))
        up = ctx.enter_context(tc.tile_pool(name="up", bufs=3))
        xbp = ctx.enter_context(tc.tile_pool(name="xbp", bufs=3))
        tqp = ctx.enter_context(tc.tile_pool(name="tqp", bufs=4))
        gp = ctx.enter_context(tc.tile_pool(name="gp", bufs=3))
        persist = ctx.enter_context(tc.tile_pool(name="persist", bufs=1))
        fin = ctx.enter_context(tc.tile_pool(name="fin", bufs=1))
        psy = ctx.enter_context(tc.tile_pool(name="psy", bufs=2, space="PSUM"))
        psx = ctx.enter_context(tc.tile_pool(name="psx", bufs=2, space="PSUM"))

        csr = cpool.tile([128, 256], F32R, tag="csr", name="csr")
        cbf = cpool.tile([128, 1427], F16, tag="cbf", name="cbf")
        cf = cpool.tile([128, 210], F32, tag="cf", name="cf")
        nc.sync.dma_start(csr[:], cs_d[:])
        nc.sync.dma_start(cbf[:], cbf_d[:])
        nc.sync.dma_start(cf[:], cf_d[:])
        twr_v = cbf[:, 0:520]
        twi_v = cbf[:, 520:1040]
        erA = cbf[:, 1040:1105]
        eipA = cbf[:, 1105:1170]
        einA = cbf[:, 1170:1235]
        ebr = cbf[:, 1235:1299]
        ebi = cbf[:, 1299:1363]
        ebin = cbf[:, 1363:1427]
        ident = cf[:, 0:128]
        mfrac = cf[:, 128:144]
        row66 = cf[:, 144:145]
        ones65 = cf[:, 145:210]

        # zero-pad the invalid tail of block 64 ([8193, 8320) = junk) and the
        # alignment pad block 65 (never gathered, but keep DRAM defined)
        zt = cpool.tile([128, 256], F16, tag="zt", name="zt")
        nc.vector.memset(zt[:], 0.0)
        nc.gpsimd.dma_start(tab_d[:, 64:65, 1:128], zt[:, 0:127])
        nc.gpsimd.dma_start(tab_d[:, 64:65, 129:256], zt[:, 0:127])
        nc.gpsimd.dma_start(tab_d[:, 65:66, :], zt[:])

        # persistent block-sum accumulators, [block, row] layout
        bsh_t = persist.tile([65, 128], F32, tag="bsh_t", name="bsh_t")
        bsc_t = persist.tile([65, 128], F32, tag="bsc_t", name="bsc_t")

        def fft_half(xg, cl, sig):
            '''Stage 1 + twiddle for chunk cl of group tile xg (one signal).
            Returns (zr, zi) fp16 [128, (R,65)] = Z[n2, k1<=64] per row.'''
            ybf = ybfp.tile([128, R * 130], F16, tag="ybf", name="ybf_" + sig)
            for half in range(2):
                y = psy.tile([128, 1024], F32, tag="y", name="y_" + sig)
                for hl in range(4):
                    rl = half * 4 + hl
                    xrow = xg[:, (cl * R + rl) * 128:(cl * R + rl + 1) * 128]
                    nc.tensor.matmul(y[:, hl * 256:(hl + 1) * 256], xrow, csr[:],
                                     start=True, stop=True)
                ysrc = y[:].rearrange("p (r t f) -> p r t f", t=2, f=128)[:, :, :, 0:65]
                nc.scalar.copy(
                    ybf[:, half * 520:(half + 1) * 520].rearrange(
                        "p (r t f) -> p r t f", t=2, f=65), ysrc)
            ycb = ybf[:].rearrange("p (r t f) -> p r t f", t=2, f=65)[:, :, 0, :]
            ysb = ybf[:].rearrange("p (r t f) -> p r t f", t=2, f=65)[:, :, 1, :]
            u1 = up.tile([128, R * 65], F16, tag="u1", name="u1_" + sig)
            u2 = up.tile([128, R * 65], F16, tag="u2", name="u2_" + sig)
            u3 = up.tile([128, R * 65], F16, tag="u3", name="u3_" + sig)
            u4 = up.tile([128, R * 65], F16, tag="u4", name="u4_" + sig)
            zr = zp.tile([128, R * 65], F16, tag="zr", name="zr_" + sig)
            zi = zp.tile([128, R * 65], F16, tag="zi", name="zi_" + sig)
            tw_r = twr_v.rearrange("p (r f) -> p r f", f=65)
            tw_i = twi_v.rearrange("p (r f) -> p r f", f=65)
            u1v = u1[:].rearrange("p (r f) -> p r f", f=65)
            u2v = u2[:].rearrange("p (r f) -> p r f", f=65)
            u3v = u3[:].rearrange("p (r f) -> p r f", f=65)
            u4v = u4[:].rearrange("p (r f) -> p r f", f=65)
            nc.vector.tensor_tensor(u1v, ycb, tw_r, op=AL.mult)
            nc.vector.tensor_tensor(u2v, ysb, tw_i, op=AL.mult)
            nc.vector.tensor_tensor(u3v, ycb, tw_i, op=AL.mult)
            nc.vector.tensor_tensor(u4v, ysb, tw_r, op=AL.mult)
            nc.vector.tensor_tensor(zi[:], u3[:], u4[:], op=AL.subtract)
            # zr = u1 + u2 is folded into stage-3 (E^T u1 + E^T u2)
            return u1, u2, zi

        def stage3(w1, w2, zi, sig):
            '''Complex stage-3 DFT over n2 with the zr-add folded in:
            zr = w1 + w2 never materializes. Returns fp16 SBUF tiles:
            (ar, ai) [65, (R,64)] for k1 in [0,64), and (br, bi) [64, (R,64)]
            whose col c maps to k1 = 127 - c.'''
            w1A = w1[:].rearrange("p (r f) -> p r f", f=65)[:, :, 0:64]
            w2A = w2[:].rearrange("p (r f) -> p r f", f=65)[:, :, 0:64]
            ziA = zi[:].rearrange("p (r f) -> p r f", f=65)[:, :, 0:64]
            w1B = w1[:].rearrange("p (r f) -> p r f", f=65)[:, :, 1:65]
            w2B = w2[:].rearrange("p (r f) -> p r f", f=65)[:, :, 1:65]
            ziB = zi[:].rearrange("p (r f) -> p r f", f=65)[:, :, 1:65]
            xr = psx.tile([65, R * 64], F32, tag="xr", name="xrA_" + sig)
            xi = psx.tile([65, R * 64], F32, tag="xi", name="xiA_" + sig)
            # grouped by stationary so consecutive loads dedup
            nc.tensor.matmul(xr[:], erA, w1A, start=True, stop=False)
            nc.tensor.matmul(xr[:], erA, w2A, start=False, stop=False)
            nc.tensor.matmul(xi[:], erA, ziA, start=True, stop=False)
            nc.tensor.matmul(xr[:], eipA, ziA, start=False, stop=True)
            nc.tensor.matmul(xi[:], einA, w1A, start=False, stop=False)
            nc.tensor.matmul(xi[:], einA, w2A, start=False, stop=True)
            ar = xbp.tile([65, R * 64], F16, tag="ar", name="arA_" + sig)
            ai = xbp.tile([65, R * 64], F16, tag="ai", name="aiA_" + sig)
            nc.scalar.mul(ar[:], xr[:], SSC)
            nc.scalar.mul(ai[:], xi[:], SSC)
            xrb = psx.tile([65, R * 64], F32, tag="xr", name="xrB_" + sig)
            xib = psx.tile([65, R * 64], F32, tag="xi", name="xiB_" + sig)
            nc.tensor.matmul(xrb[0:64, :], ebr, w1B, start=True, stop=False)
            nc.tensor.matmul(xrb[0:64, :], ebr, w2B, start=False, stop=False)
            nc.tensor.matmul(xib[0:64, :], ebr, ziB, start=True, stop=False)
            nc.tensor.matmul(xrb[0:64, :], ebin, ziB, start=False, stop=True)
            nc.tensor.matmul(xib[0:64, :], ebi, w1B, start=False, stop=False)
            nc.tensor.matmul(xib[0:64, :], ebi, w2B, start=False, stop=True)
            br = xbp.tile([65, R * 64], F16, tag="br", name="brB_" + sig)
            bi = xbp.tile([65, R * 64], F16, tag="bi", name="biB_" + sig)
            nc.scalar.mul(br[0:64, :], xrb[0:64, :], SSC)
            nc.scalar.mul(bi[0:64, :], xib[0:64, :], SSC)
            return ar, ai, br, bi

        def pair_prod_sum(dest, e0, e1, f0, f1, parts, rev, eng_p, eng_a):
            '''dest = e0*e1 + f0*f1 elementwise (fp16), written through
            3D views; rev reverses the c-axis of the sources (set B).'''
            t1 = tqp.tile([65, R * 64], F16, tag="t1", name="pp1")
            t2 = tqp.tile([65, R * 64], F16, tag="t2", name="pp2")
            nc.vector.tensor_tensor(t1[0:parts, :], e0[0:parts, :],
                                    e1[0:parts, :], op=AL.mult)
            eng_p.tensor_tensor(t2[0:parts, :], f0[0:parts, :],
                                f1[0:parts, :], op=AL.mult)
            if rev:
                s1 = t1[0:parts, :].rearrange("p (r f) -> p r f", f=64)[:, :, 63::-1]
                s2 = t2[0:parts, :].rearrange("p (r f) -> p r f", f=64)[:, :, 63::-1]
            else:
                s1 = t1[0:parts, :].rearrange("p (r f) -> p r f", f=64)
                s2 = t2[0:parts, :].rearrange("p (r f) -> p r f", f=64)
            eng_a.tensor_tensor(dest, s1, s2, op=AL.add)

        def load_group(g):
            xg_t = inp.tile([128, GR * 128], F32R, tag="xg_t", name="xg_t")
            xg_s = inp.tile([128, GR * 128], F32R, tag="xg_s", name="xg_s")
            nc.sync.dma_start(
                xg_t[:].rearrange("p (r f) -> p r f", r=GR),
                t_in[g * GR:(g + 1) * GR, :].rearrange("r (p f) -> p r f", p=128))
            nc.sync.dma_start(
                xg_s[:].rearrange("p (r f) -> p r f", r=GR),
                s_in[g * GR:(g + 1) * GR, :].rearrange("r (p f) -> p r f", p=128))
            return xg_t, xg_s

        pending = load_group(0)
        for g in range(GROUPS):
            xg_t, xg_s = pending
            if g + 1 < GROUPS:
                pending = load_group(g + 1)
            gph = gp.tile([65, GR * 128], F16, tag="gph", name="gph")
            gcr = gp.tile([65, GR * 128], F16, tag="gcr", name="gcr")
            gph3 = gph[:].rearrange("p (r f) -> p r f", f=128)
            gcr3 = gcr[:].rearrange("p (r f) -> p r f", f=128)
            for cl in range(CPG):
                ci = g * CPG + cl           # global chunk id
                u1_t, u2_t, zi_t = fft_half(xg_t, cl, "t")
                u1_s, u2_s, zi_s = fft_half(xg_s, cl, "s")
                ar_t, ai_t, br_t, bi_t = stage3(u1_t, u2_t, zi_t, "t")
                ar_s, ai_s, br_s, bi_s = stage3(u1_s, u2_s, zi_s, "s")
                r0, r1 = cl * R, (cl + 1) * R
                # ph = |X_t|^2 ; cr = Re(conj(X_t) X_s)  (both c-scaled)
                pair_prod_sum(gph3[:, r0:r1, 0:64], ar_t, ar_t, ai_t, ai_t,
                              65, False, nc.vector, nc.vector)
                pair_prod_sum(gph3[0:64, r0:r1, 64:128], br_t, br_t, bi_t, bi_t,
                              64, True, nc.gpsimd, nc.vector)
                pair_prod_sum(gcr3[:, r0:r1, 0:64], ar_t, ar_s, ai_t, ai_s,
                              65, False, nc.gpsimd, nc.gpsimd)
                pair_prod_sum(gcr3[0:64, r0:r1, 64:128], br_t, br_s, bi_t, bi_s,
                              64, True, nc.gpsimd, nc.gpsimd)
                # block sums (full blocks 0..63); tail block = single element
                nc.vector.tensor_reduce(
                    bsh_t[0:64, ci * R:(ci + 1) * R], gph3[0:64, r0:r1, :],
                    op=AL.add, axis=AX.X)
                nc.vector.tensor_reduce(
                    bsc_t[0:64, ci * R:(ci + 1) * R], gcr3[0:64, r0:r1, :],
                    op=AL.add, axis=AX.X)
                nc.scalar.copy(bsh_t[64:65, ci * R:(ci + 1) * R],
                               gph[64:65, r0 * 128:r1 * 128:128])
                nc.scalar.copy(bsc_t[64:65, ci * R:(ci + 1) * R],
                               gcr[64:65, r0 * 128:r1 * 128:128])
            # stage this group's ph/cr to DRAM, row-major with pitch 66 blocks
            nc.gpsimd.dma_start(
                tab_d[g * GR:(g + 1) * GR, 0:64, 0:128].rearrange("r b j -> b r j"),
                gph[0:64, :].rearrange("p (r j) -> p r j", j=128))
            nc.gpsimd.dma_start(
                tab_d[g * GR:(g + 1) * GR, 0:64, 128:256].rearrange("r b j -> b r j"),
                gcr[0:64, :].rearrange("p (r j) -> p r j", j=128))

        # ---- tail-block (k = 8192) values to DRAM ----
        tbh = fin.tile([1, 128], F16, tag="tbh", name="tbh")
        tbc = fin.tile([1, 128], F16, tag="tbc", name="tbc")
        nc.vector.tensor_copy(tbh[:], bsh_t[64:65, :])
        nc.vector.tensor_copy(tbc[:], bsc_t[64:65, :])
        nc.gpsimd.dma_start(tab_d[:, 64:65, 0:1].rearrange("r b j -> b r j"), tbh[:])
        nc.gpsimd.dma_start(tab_d[:, 64:65, 128:129].rearrange("r b j -> b r j"), tbc[:])

        # ---- transpose block sums to [row, block] ----
        trp = psy.tile([128, 1024], F32, tag="y", name="tr_ps")
        nc.tensor.transpose(trp[:, 0:65], bsh_t[:], ident[0:65, 0:65])
        nc.tensor.transpose(trp[:, 512:577], bsc_t[:], ident[0:65, 0:65])
        bsh = fin.tile([128, 65], F32, tag="bsh", name="bsh")
        bsc = fin.tile([128, 65], F32, tag="bsc", name="bsc")
        nc.scalar.copy(bsh[:], trp[:, 0:65])
        nc.scalar.copy(bsc[:], trp[:, 512:577])
        chb = fin.tile([128, 65], F32, tag="chb", name="chb")
        cqb = fin.tile([128, 65], F32, tag="cqb", name="cqb")
        nc.vector.tensor_tensor_scan(chb[:], bsh[:], bsh[:], 0.0,
                                     op0=AL.add, op1=AL.bypass)
        nc.vector.tensor_tensor_scan(cqb[:], bsc[:], bsc[:], 0.0,
                                     op0=AL.add, op1=AL.bypass)
        th = chb[:, 64:65]     # total_h
        tc_ = cqb[:, 64:65]    # total_c
        tvals = fin.tile([128, NM], F32, tag="tvals", name="tvals")
        nc.vector.tensor_scalar(tvals[:], mfrac, th, None, op0=AL.mult)

        # ---- coarse masked sums over blocks ----
        bstar = fin.tile([128, NM], F32, tag="bstar", name="bstar")
        acc_a = fin.tile([128, NM], F32, tag="acc_a", name="acc_a")
        acc_p = fin.tile([128, NM], F32, tag="acc_p", name="acc_p")
        junk_v = fin.tile([128, 65], F32, tag="junk_v", name="junk_v")
        junk_g = fin.tile([128, 65], F32, tag="junk_g", name="junk_g")
        idxf = fin.tile([128, NM], F32, tag="idxf", name="idxf")
        idx = fin.tile([128, NM], I32, tag="idx", name="idx")
        wins = fin.tile([128, NM * 256], F16, tag="wins", name="wins")
        tab_flat = tab_d[:].rearrange("r b j -> (r b) j")
        # m = 0: t_0 = 0 so the straddling block is always block 0 — fetch it
        # with a direct DMA on the SP queue, off the serial gather chain and
        # independent of the index computation.
        nc.sync.dma_start(wins[:, 0:256],
                          tab_d[:, 0:1, :].rearrange("r b j -> r (b j)"))
        # Coarse pass per threshold; each gather issues as soon as its own
        # index column is ready so the serial SWDGE chain starts early.
        for m in range(NM):
            sc = tvals[:, m:m + 1]
            nc.vector.scalar_tensor_tensor(
                junk_v[:], chb[:], sc, ones65, op0=AL.is_le, op1=AL.mult,
                accum_out=bstar[:, m:m + 1])
            if m >= 1:
                nc.vector.tensor_scalar(idxf[:, m:m + 1], bstar[:, m:m + 1],
                                        row66, None, op0=AL.add)
                nc.vector.tensor_copy(idx[:, m:m + 1], idxf[:, m:m + 1])
                nc.gpsimd.indirect_dma_start(
                    wins[:, m * 256:(m + 1) * 256], None, tab_flat,
                    bass.IndirectOffsetOnAxis(ap=idx[:, m:m + 1], axis=0))
            nc.vector.scalar_tensor_tensor(
                junk_g[:], chb[:], sc, bsc[:], op0=AL.is_le, op1=AL.mult,
                accum_out=acc_a[:, m:m + 1])
            nc.vector.scalar_tensor_tensor(
                junk_g[:], chb[:], sc, bsh[:], op0=AL.is_le, op1=AL.mult,
                accum_out=acc_p[:, m:m + 1])
        tau = fin.tile([128, NM], F32, tag="tau", name="tau")
        nc.vector.tensor_tensor(tau[:], tvals[:], acc_p[:], op=AL.subtract)

        # ---- fine correction: F_m = sum_j [CHprev <= t] cr within block ----
        loc = fin.tile([128, NM * 128], F32, tag="loc", name="loc")
        f1 = fin.tile([128, NM], F32, tag="f1", name="f1")
        junk2 = fin.tile([128, 127], F32, tag="junk2", name="junk2")
        for m in range(NM):
            phw = wins[:, m * 256:m * 256 + 128]
            crw = wins[:, m * 256 + 128:m * 256 + 256]
            sl = slice(m * 128, (m + 1) * 128)
            nc.vector.tensor_tensor_scan(
                loc[:, sl], phw, phw, 0.0, op0=AL.add, op1=AL.bypass)
            nc.vector.scalar_tensor_tensor(
                junk2[:], loc[:, m * 128:m * 128 + 127], tau[:, m:m + 1],
                crw[:, 1:128], op0=AL.is_le, op1=AL.mult,
                accum_out=f1[:, m:m + 1])
        crw0 = fin.tile([128, NM], F32, tag="crw0", name="crw0")
        nc.vector.tensor_copy(crw0[:], wins[:, 128:NM * 256:256])

        # negG[m] = A_m + F_m - total_c  (so snr_bin = negG[m+1] - negG[m])
        negg = fin.tile([128, NM + 1], F32, tag="negg", name="negg")
        nc.vector.memset(negg[:, NM:NM + 1], 0.0)
        nc.vector.tensor_tensor(negg[:, 0:NM], acc_a[:], f1[:], op=AL.add)
        nc.vector.tensor_tensor(negg[:, 0:NM], negg[:, 0:NM], crw0[:], op=AL.add)
        nc.vector.tensor_scalar(negg[:, 0:NM], negg[:, 0:NM], tc_, None,
                                op0=AL.subtract)
        snr = fin.tile([128, NM], F32, tag="snr", name="snr")
        nc.vector.tensor_tensor(snr[:], negg[:, 1:NM + 1], negg[:, 0:NM],
                                op=AL.subtract)
        s16 = fin.tile([128, 1], F32, tag="s16", name="s16")
        nc.vector.tensor_scalar_mul(s16[:], tc_, 1.0 / NUM_BINS)
        ee = fin.tile([128, NM], F32, tag="ee", name="ee")
        nc.vector.tensor_scalar(ee[:], snr[:], s16[:], None, op0=AL.subtract)
        esq = fin.tile([128, NM], F32, tag="esq", name="esq")
        nc.vector.tensor_tensor(esq[:], ee[:], ee[:], op=AL.mult)
        ssum = fin.tile([128, 1], F32, tag="ssum", name="ssum")
        nc.vector.tensor_reduce(ssum[:], esq[:], op=AL.add, axis=AX.X)
        rth = fin.tile([128, 1], F32, tag="rth", name="rth")
        nc.vector.reciprocal(rth[:], th)
        chq = fin.tile([128, 1], F32, tag="chq", name="chq")
        nc.vector.tensor_tensor(chq[:], ssum[:], rth[:], op=AL.mult)
        nc.vector.tensor_scalar_mul(chq[:], chq[:],
                                    float(NUM_BINS) / (NUM_BINS - 1))
        nc.sync.dma_start(out[:], chq[:])

    nc.compile()
    return nc, _make_consts()


_CACHE = {}


def kernel(template: np.ndarray, strain: np.ndarray) -> np.ndarray:
    if "nc" not in _CACHE:
        _CACHE["nc"], _CACHE["consts"] = _build_program()
    nc, consts = _CACHE["nc"], _CACHE["consts"]

    t = np.ascontiguousarray(np.asarray(template, np.float32).reshape(1024, N))
    s = np.ascontiguousarray(np.asarray(strain, np.float32).reshape(1024, N))
    in_maps = []
    for c in range(NCORES):
        m = {"t_in": t[c * ROWS:(c + 1) * ROWS],
             "s_in": s[c * ROWS:(c + 1) * ROWS]}
        m.update(consts)
        in_maps.append(m)
    res = run_bass_kernel_spmd(nc, in_maps, list(range(NCORES)))
    outs = [res.results[c]["chisq_out"].reshape(ROWS) for c in range(NCORES)]
    full = np.concatenate(outs).astype(np.float32)
    return full.reshape(512, 2)


if __name__ == "__main__":
    rng = np.random.default_rng(0)
    tpl = rng.standard_normal((512, 2, N), dtype=np.float32)
    st = rng.standard_normal((512, 2, N), dtype=np.float32)
    print(kernel(tpl, st)[:3])


# revision 48
# speedup vs baseline: 1.0306x; 1.0306x over previous
"""Trainium2 Bass kernel for the ChiSq (histogram_binning) problem.

Per core (128 of 1024 rows, pure data parallel):
  FFT-16384 of template & strain via 2-stage radix-128 matmul FFT with the
  data as the stationary operand (no transposes), Hermitian-halved twiddle
  (k1 <= 64 only; the upper half of each 128-block is reconstructed inside
  stage-3 via a conjugated E-matrix "set B" whose outputs are written
  reversed), fp32r stage-1 and fp16 moving operands for full-rate PE.
  Binning is hierarchical: per-128 block sums -> block-level masked sums on
  [128, 65] arrays, plus one indirect-DMA gather per threshold of the single
  straddling block (HW DGE streams consecutive rows from one offset per
  partition) for the exact fine correction. This avoids both the row-major
  unflatten DMAs and the 15 full-array masked passes of the naive approach.
  Elementwise work is fp16 (2x DVE), split across DVE/GpSimd; PSUM->fp16
  converts (with the CSCALE fold) run on the Activation engine.
"""
import numpy as np
from contextlib import ExitStack

import concourse.bass as bass
import concourse.tile as tile
from concourse import bacc, mybir
from concourse.bass_utils import run_bass_kernel_spmd

F32 = mybir.dt.float32
F32R = mybir.dt.float32r
BF16 = mybir.dt.bfloat16
F16 = mybir.dt.float16
I32 = mybir.dt.int32

SAMPLE_RATE = 2048.0
FFTLENGTH = 8.0
NUM_BINS = 16
N = int(FFTLENGTH * SAMPLE_RATE)       # 16384
NF = N // 2 + 1                        # 8193
DF = 1.0 / FFTLENGTH
CSCALE = 4.0 * DF / (SAMPLE_RATE ** 2)

NCORES = 8
ROWS = 128          # rows per core
GROUPS = 8
GR = 16             # rows per group (DMA granularity)
R = 8               # rows per compute chunk
CPG = GR // R       # chunks per group
NB = 65             # 64 full blocks + tail block (k = 8192)
PITCH = 66          # DRAM row pitch in 128-blocks (alignment pad)
NM = 16             # thresholds m = 0..15 (t_0 = 0)
SSC = float(np.sqrt(CSCALE))   # folded into the stage-3 PSUM->fp16 converts


def _make_consts():
    s = np.sqrt(np.float32(CSCALE))
    n1 = np.arange(128)
    k1 = np.arange(128)
    ang1 = 2 * np.pi * np.outer(n1, k1) / 128.0
    CS = np.concatenate([np.cos(ang1), np.sin(ang1)], axis=1).astype(np.float32)

    n2 = np.arange(128)
    j = np.arange(65)
    angw = 2 * np.pi * np.outer(n2, j) / float(N)
    twr = np.cos(angw)
    twi = -np.sin(angw)
    twr_rep = np.tile(twr, (1, R))    # [128, 260] (r, j) blocks
    twi_rep = np.tile(twi, (1, R))

    k2 = np.arange(65)
    angA = 2 * np.pi * np.outer(n2, k2) / 128.0
    erA = np.cos(angA)
    eipA = np.sin(angA)
    einA = -eipA
    k2b = np.arange(64)
    angB = 2 * np.pi * np.outer(n2, k2b + 1) / 128.0
    ebr = np.cos(angB)
    ebi = np.sin(angB)
    ebin = -ebi
    cbf = np.concatenate(
        [twr_rep, twi_rep, erA, eipA, einA, ebr, ebi, ebin], axis=1)  # [128, 1427]

    ident = np.eye(128, dtype=np.float32)
    mfrac = np.tile((np.arange(16, dtype=np.float32) / 16.0)[None, :], (128, 1))
    row66 = (np.arange(128, dtype=np.float32) * PITCH)[:, None]
    ones65 = np.ones((128, 65), dtype=np.float32)
    cf = np.concatenate([ident, mfrac, row66, ones65], axis=1)  # [128, 210]

    return dict(
        cs=np.ascontiguousarray(CS, np.float32),
        cbf=np.ascontiguousarray(cbf).astype(np.float16),
        cf=np.ascontiguousarray(cf, np.float32),
    )


def _build_program():
    nc = bacc.Bacc("TRN2", target_bir_lowering=False, debug=False,
                   enable_asserts=False, num_devices=NCORES)
    t_in = nc.dram_tensor("t_in", [ROWS, N], F32R, kind="ExternalInput").ap()
    s_in = nc.dram_tensor("s_in", [ROWS, N], F32R, kind="ExternalInput").ap()
    cs_d = nc.dram_tensor("cs", [128, 256], F32R, kind="ExternalInput").ap()
    cbf_d = nc.dram_tensor("cbf", [128, 1427], F16, kind="ExternalInput").ap()
    cf_d = nc.dram_tensor("cf", [128, 210], F32, kind="ExternalInput").ap()
    # interleaved staging table: [..., 0:128] = ph block, [..., 128:256] = cr
    tab_d = nc.dram_tensor("tab_d", [ROWS, PITCH, 256], F16, kind="Internal").ap()
    out = nc.dram_tensor("chisq_out", [ROWS, 1], F32, kind="ExternalOutput").ap()

    AL = mybir.AluOpType
    AX = mybir.AxisListType

    with tile.TileContext(nc, trace_sim=False) as tc, ExitStack() as ctx:
        cpool = ctx.enter_context(tc.tile_pool(name="consts", bufs=1))
        inp = ctx.enter_context(tc.tile_pool(name="inp", bufs=3))
        ybfp = ctx.enter_context(tc.tile_pool(name="ybfp", bufs=3))
        zp = ctx.enter_context(tc.tile_pool(name="zp", bufs=3))
        up = ctx.enter_context(tc.tile_pool(name="up", bufs=3))
        xbp = ctx.enter_context(tc.tile_pool(name="xbp", bufs=3))
        tqp = ctx.enter_context(tc.tile_pool(name="tqp", bufs=4))
        gp = ctx.enter_context(tc.tile_pool(name="gp", bufs=3))
        persist = ctx.enter_context(tc.tile_pool(name="persist", bufs=1))
        fin = ctx.enter_context(tc.tile_pool(name="fin", bufs=1))
        psy = ctx.enter_context(tc.tile_pool(name="psy", bufs=2, space="PSUM"))
        psx = ctx.enter_context(tc.tile_pool(name="psx", bufs=2, space="PSUM"))

        csr = cpool.tile([128, 256], F32R, tag="csr", name="csr")
        cbf = cpool.tile([128, 1427], F16, tag="cbf", name="cbf")
        cf = cpool.tile([128, 210], F32, tag="cf", name="cf")
        nc.sync.dma_start(csr[:], cs_d[:])
        nc.sync.dma_start(cbf[:], cbf_d[:])
        nc.sync.dma_start(cf[:], cf_d[:])
        twr_v = cbf[:, 0:520]
        twi_v = cbf[:, 520:1040]
        erA = cbf[:, 1040:1105]
        eipA = cbf[:, 1105:1170]
        einA = cbf[:, 1170:1235]
        ebr = cbf[:, 1235:1299]
        ebi = cbf[:, 1299:1363]
        ebin = cbf[:, 1363:1427]
        ident = cf[:, 0:128]
        mfrac = cf[:, 128:144]
        row66 = cf[:, 144:145]
        ones65 = cf[:, 145:210]

        # zero-pad the invalid tail of block 64 ([8193, 8320) = junk) and the
        # alignment pad block 65 (never gathered, but keep DRAM defined)
        zt = cpool.tile([128, 256], F16, tag="zt", name="zt")
        nc.vector.memset(zt[:], 0.0)
        nc.gpsimd.dma_start(tab_d[:, 64:65, 1:128], zt[:, 0:127])
        nc.gpsimd.dma_start(tab_d[:, 64:65, 129:256], zt[:, 0:127])
        nc.gpsimd.dma_start(tab_d[:, 65:66, :], zt[:])

        # persistent block-sum accumulators, [block, row] layout
        bsh_t = persist.tile([65, 128], F32, tag="bsh_t", name="bsh_t")
        bsc_t = persist.tile([65, 128], F32, tag="bsc_t", name="bsc_t")

        def fft_half(xg, cl, sig):
            '''Stage 1 + twiddle for chunk cl of group tile xg (one signal).
            Returns (zr, zi) fp16 [128, (R,65)] = Z[n2, k1<=64] per row.'''
            ybf = ybfp.tile([128, R * 130], F16, tag="ybf", name="ybf_" + sig)
            for half in range(2):
                y = psy.tile([128, 1024], F32, tag="y", name="y_" + sig)
                for hl in range(4):
                    rl = half * 4 + hl
                    xrow = xg[:, (cl * R + rl) * 128:(cl * R + rl + 1) * 128]
                    nc.tensor.matmul(y[:, hl * 256:(hl + 1) * 256], xrow, csr[:],
                                     start=True, stop=True)
                ysrc = y[:].rearrange("p (r t f) -> p r t f", t=2, f=128)[:, :, :, 0:65]
                nc.scalar.copy(
                    ybf[:, half * 520:(half + 1) * 520].rearrange(
                        "p (r t f) -> p r t f", t=2, f=65), ysrc)
            ycb = ybf[:].rearrange("p (r t f) -> p r t f", t=2, f=65)[:, :, 0, :]
            ysb = ybf[:].rearrange("p (r t f) -> p r t f", t=2, f=65)[:, :, 1, :]
            u1 = up.tile([128, R * 65], F16, tag="u1", name="u1_" + sig)
            u2 = up.tile([128, R * 65], F16, tag="u2", name="u2_" + sig)
            u3 = up.tile([128, R * 65], F16, tag="u3", name="u3_" + sig)
            u4 = up.tile([128, R * 65], F16, tag="u4", name="u4_" + sig)
            zr = zp.tile([128, R * 65], F16, tag="zr", name="zr_" + sig)
            zi = zp.tile([128, R * 65], F16, tag="zi", name="zi_" + sig)
            tw_r = twr_v.rearrange("p (r f) -> p r f", f=65)
            tw_i = twi_v.rearrange("p (r f) -> p r f", f=65)
            u1v = u1[:].rearrange("p (r f) -> p r f", f=65)
            u2v = u2[:].rearrange("p (r f) -> p r f", f=65)
            u3v = u3[:].rearrange("p (r f) -> p r f", f=65)
            u4v = u4[:].rearrange("p (r f) -> p r f", f=65)
            nc.vector.tensor_tensor(u1v, ycb, tw_r, op=AL.mult)
            nc.vector.tensor_tensor(u2v, ysb, tw_i, op=AL.mult)
            nc.vector.tensor_tensor(u3v, ycb, tw_i, op=AL.mult)
            nc.vector.tensor_tensor(u4v, ysb, tw_r, op=AL.mult)
            nc.vector.tensor_tensor(zi[:], u3[:], u4[:], op=AL.subtract)
            # zr = u1 + u2 is folded into stage-3 (E^T u1 + E^T u2)
            return u1, u2, zi

        def stage3(w1, w2, zi, sig):
            '''Complex stage-3 DFT over n2 with the zr-add folded in:
            zr = w1 + w2 never materializes. Returns fp16 SBUF tiles:
            (ar, ai) [65, (R,64)] for k1 in [0,64), and (br, bi) [64, (R,64)]
            whose col c maps to k1 = 127 - c.'''
            w1A = w1[:].rearrange("p (r f) -> p r f", f=65)[:, :, 0:64]
            w2A = w2[:].rearrange("p (r f) -> p r f", f=65)[:, :, 0:64]
            ziA = zi[:].rearrange("p (r f) -> p r f", f=65)[:, :, 0:64]
            w1B = w1[:].rearrange("p (r f) -> p r f", f=65)[:, :, 1:65]
            w2B = w2[:].rearrange("p (r f) -> p r f", f=65)[:, :, 1:65]
            ziB = zi[:].rearrange("p (r f) -> p r f", f=65)[:, :, 1:65]
            xr = psx.tile([65, R * 64], F32, tag="xr", name="xrA_" + sig)
            xi = psx.tile([65, R * 64], F32, tag="xi", name="xiA_" + sig)
            # grouped by stationary so consecutive loads dedup
            nc.tensor.matmul(xr[:], erA, w1A, start=True, stop=False)
            nc.tensor.matmul(xr[:], erA, w2A, start=False, stop=False)
            nc.tensor.matmul(xi[:], erA, ziA, start=True, stop=False)
            nc.tensor.matmul(xr[:], eipA, ziA, start=False, stop=True)
            nc.tensor.matmul(xi[:], einA, w1A, start=False, stop=False)
            nc.tensor.matmul(xi[:], einA, w2A, start=False, stop=True)
            ar = xbp.tile([65, R * 64], F16, tag="ar", name="arA_" + sig)
            ai = xbp.tile([65, R * 64], F16, tag="ai", name="aiA_" + sig)
            nc.scalar.mul(ar[:], xr[:], SSC)
            nc.scalar.mul(ai[:], xi[:], SSC)
            xrb = psx.tile([65, R * 64], F32, tag="xr", name="xrB_" + sig)
            xib = psx.tile([65, R * 64], F32, tag="xi", name="xiB_" + sig)
            nc.tensor.matmul(xrb[0:64, :], ebr, w1B, start=True, stop=False)
            nc.tensor.matmul(xrb[0:64, :], ebr, w2B, start=False, stop=False)
            nc.tensor.matmul(xib[0:64, :], ebr, ziB, start=True, stop=False)
            nc.tensor.matmul(xrb[0:64, :], ebin, ziB, start=False, stop=True)
            nc.tensor.matmul(xib[0:64, :], ebi, w1B, start=False, stop=False)
            nc.tensor.matmul(xib[0:64, :], ebi, w2B, start=False, stop=True)
            br = xbp.tile([65, R * 64], F16, tag="br", name="brB_" + sig)
            bi = xbp.tile([65, R * 64], F16, tag="bi", name="biB_" + sig)
            nc.scalar.mul(br[0:64, :], xrb[0:64, :], SSC)
            nc.scalar.mul(bi[0:64, :], xib[0:64, :], SSC)
            return ar, ai, br, bi

        def pair_prod_sum(dest, e0, e1, f0, f1, parts, rev, eng_p, eng_a):
            '''dest = e0*e1 + f0*f1 elementwise (fp16), written through
            3D views; rev reverses the c-axis of the sources (set B).'''
            t1 = tqp.tile([65, R * 64], F16, tag="t1", name="pp1")
            t2 = tqp.tile([65, R * 64], F16, tag="t2", name="pp2")
            nc.vector.tensor_tensor(t1[0:parts, :], e0[0:parts, :],
                                    e1[0:parts, :], op=AL.mult)
            eng_p.tensor_tensor(t2[0:parts, :], f0[0:parts, :],
                                f1[0:parts, :], op=AL.mult)
            if rev:
                s1 = t1[0:parts, :].rearrange("p (r f) -> p r f", f=64)[:, :, 63::-1]
                s2 = t2[0:parts, :].rearrange("p (r f) -> p r f", f=64)[:, :, 63::-1]
            else:
                s1 = t1[0:parts, :].rearrange("p (r f) -> p r f", f=64)
                s2 = t2[0:parts, :].rearrange("p (r f) -> p r f", f=64)
            eng_a.tensor_tensor(dest, s1, s2, op=AL.add)

        def load_group(g):
            xg_t = inp.tile([128, GR * 128], F32R, tag="xg_t", name="xg_t")
            xg_s = inp.tile([128, GR * 128], F32R, tag="xg_s", name="xg_s")
            nc.sync.dma_start(
                xg_t[:].rearrange("p (r f) -> p r f", r=GR),
                t_in[g * GR:(g + 1) * GR, :].rearrange("r (p f) -> p r f", p=128))
            nc.sync.dma_start(
                xg_s[:].rearrange("p (r f) -> p r f", r=GR),
                s_in[g * GR:(g + 1) * GR, :].rearrange("r (p f) -> p r f", p=128))
            return xg_t, xg_s

        pending = load_group(0)
        for g in range(GROUPS):
            xg_t, xg_s = pending
            if g + 1 < GROUPS:
                pending = load_group(g + 1)
            gph = gp.tile([65, GR * 128], F16, tag="gph", name="gph")
            gcr = gp.tile([65, GR * 128], F16, tag="gcr", name="gcr")
            gph3 = gph[:].rearrange("p (r f) -> p r f", f=128)
            gcr3 = gcr[:].rearrange("p (r f) -> p r f", f=128)
            for cl in range(CPG):
                ci = g * CPG + cl           # global chunk id
                u1_t, u2_t, zi_t = fft_half(xg_t, cl, "t")
                u1_s, u2_s, zi_s = fft_half(xg_s, cl, "s")
                ar_t, ai_t, br_t, bi_t = stage3(u1_t, u2_t, zi_t, "t")
                ar_s, ai_s, br_s, bi_s = stage3(u1_s, u2_s, zi_s, "s")
                r0, r1 = cl * R, (cl + 1) * R
                # ph = |X_t|^2 ; cr = Re(conj(X_t) X_s)  (both c-scaled)
                pair_prod_sum(gph3[:, r0:r1, 0:64], ar_t, ar_t, ai_t, ai_t,
                              65, False, nc.vector, nc.vector)
                pair_prod_sum(gph3[0:64, r0:r1, 64:128], br_t, br_t, bi_t, bi_t,
                              64, True, nc.gpsimd, nc.vector)
                pair_prod_sum(gcr3[:, r0:r1, 0:64], ar_t, ar_s, ai_t, ai_s,
                              65, False, nc.gpsimd, nc.vector)
                pair_prod_sum(gcr3[0:64, r0:r1, 64:128], br_t, br_s, bi_t, bi_s,
                              64, True, nc.gpsimd, nc.gpsimd)
                # block sums (full blocks 0..63); tail block = single element
                nc.vector.tensor_reduce(
                    bsh_t[0:64, ci * R:(ci + 1) * R], gph3[0:64, r0:r1, :],
                    op=AL.add, axis=AX.X)
                nc.vector.tensor_reduce(
                    bsc_t[0:64, ci * R:(ci + 1) * R], gcr3[0:64, r0:r1, :],
                    op=AL.add, axis=AX.X)
                nc.scalar.copy(bsh_t[64:65, ci * R:(ci + 1) * R],
                               gph[64:65, r0 * 128:r1 * 128:128])
                nc.scalar.copy(bsc_t[64:65, ci * R:(ci + 1) * R],
                               gcr[64:65, r0 * 128:r1 * 128:128])
            # stage this group's ph/cr to DRAM, row-major with pitch 66 blocks
            nc.gpsimd.dma_start(
                tab_d[g * GR:(g + 1) * GR, 0:64, 0:128].rearrange("r b j -> b r j"),
                gph[0:64, :].rearrange("p (r j) -> p r j", j=128))
            nc.gpsimd.dma_start(
                tab_d[g * GR:(g + 1) * GR, 0:64, 128:256].rearrange("r b j -> b r j"),
                gcr[0:64, :].rearrange("p (r j) -> p r j", j=128))

        # ---- tail-block (k = 8192) values to DRAM ----
        tbh = fin.tile([1, 128], F16, tag="tbh", name="tbh")
        tbc = fin.tile([1, 128], F16, tag="tbc", name="tbc")
        nc.vector.tensor_copy(tbh[:], bsh_t[64:65, :])
        nc.vector.tensor_copy(tbc[:], bsc_t[64:65, :])
        nc.gpsimd.dma_start(tab_d[:, 64:65, 0:1].rearrange("r b j -> b r j"), tbh[:])
        nc.gpsimd.dma_start(tab_d[:, 64:65, 128:129].rearrange("r b j -> b r j"), tbc[:])

        # ---- transpose block sums to [row, block] ----
        trp = psy.tile([128, 1024], F32, tag="y", name="tr_ps")
        nc.tensor.transpose(trp[:, 0:65], bsh_t[:], ident[0:65, 0:65])
        nc.tensor.transpose(trp[:, 512:577], bsc_t[:], ident[0:65, 0:65])
        bsh = fin.tile([128, 65], F32, tag="bsh", name="bsh")
        bsc = fin.tile([128, 65], F32, tag="bsc", name="bsc")
        nc.scalar.copy(bsh[:], trp[:, 0:65])
        nc.scalar.copy(bsc[:], trp[:, 512:577])
        chb = fin.tile([128, 65], F32, tag="chb", name="chb")
        cqb = fin.tile([128, 65], F32, tag="cqb", name="cqb")
        nc.vector.tensor_tensor_scan(chb[:], bsh[:], bsh[:], 0.0,
                                     op0=AL.add, op1=AL.bypass)
        nc.vector.tensor_tensor_scan(cqb[:], bsc[:], bsc[:], 0.0,
                                     op0=AL.add, op1=AL.bypass)
        th = chb[:, 64:65]     # total_h
        tc_ = cqb[:, 64:65]    # total_c
        tvals = fin.tile([128, NM], F32, tag="tvals", name="tvals")
        nc.vector.tensor_scalar(tvals[:], mfrac, th, None, op0=AL.mult)

        # ---- coarse masked sums over blocks ----
        bstar = fin.tile([128, NM], F32, tag="bstar", name="bstar")
        acc_a = fin.tile([128, NM], F32, tag="acc_a", name="acc_a")
        acc_p = fin.tile([128, NM], F32, tag="acc_p", name="acc_p")
        junk_v = fin.tile([128, 65], F32, tag="junk_v", name="junk_v")
        junk_g = fin.tile([128, 65], F32, tag="junk_g", name="junk_g")
        idxf = fin.tile([128, NM], F32, tag="idxf", name="idxf")
        idx = fin.tile([128, NM], I32, tag="idx", name="idx")
        wins = fin.tile([128, NM * 256], F16, tag="wins", name="wins")
        tab_flat = tab_d[:].rearrange("r b j -> (r b) j")
        # m = 0: t_0 = 0 so the straddling block is always block 0 — fetch it
        # with a direct DMA on the SP queue, off the serial gather chain and
        # independent of the index computation.
        nc.sync.dma_start(wins[:, 0:256],
                          tab_d[:, 0:1, :].rearrange("r b j -> r (b j)"))
        # Coarse pass per threshold; each gather issues as soon as its own
        # index column is ready so the serial SWDGE chain starts early.
        for m in range(NM):
            sc = tvals[:, m:m + 1]
            nc.vector.scalar_tensor_tensor(
                junk_v[:], chb[:], sc, ones65, op0=AL.is_le, op1=AL.mult,
                accum_out=bstar[:, m:m + 1])
            if m >= 1:
                nc.vector.tensor_scalar(idxf[:, m:m + 1], bstar[:, m:m + 1],
                                        row66, None, op0=AL.add)
                nc.vector.tensor_copy(idx[:, m:m + 1], idxf[:, m:m + 1])
                nc.gpsimd.indirect_dma_start(
                    wins[:, m * 256:(m + 1) * 256], None, tab_flat,
                    bass.IndirectOffsetOnAxis(ap=idx[:, m:m + 1], axis=0))
            nc.vector.scalar_tensor_tensor(
                junk_g[:], chb[:], sc, bsc[:], op0=AL.is_le, op1=AL.mult,
                accum_out=acc_a[:, m:m + 1])
            nc.vector.scalar_tensor_tensor(
                junk_g[:], chb[:], sc, bsh[:], op0=AL.is_le, op1=AL.mult,
                accum_out=acc_p[:, m:m + 1])
        tau = fin.tile([128, NM], F32, tag="tau", name="tau")
        nc.vector.tensor_tensor(tau[:], tvals[:], acc_p[:], op=AL.subtract)

        # ---- fine correction: F_m = sum_j [CHprev <= t] cr within block ----
        loc = fin.tile([128, NM * 128], F32, tag="loc", name="loc")
        f1 = fin.tile([128, NM], F32, tag="f1", name="f1")
        junk2 = fin.tile([128, 127], F32, tag="junk2", name="junk2")
        for m in range(NM):
            phw = wins[:, m * 256:m * 256 + 128]
            crw = wins[:, m * 256 + 128:m * 256 + 256]
            sl = slice(m * 128, (m + 1) * 128)
            nc.vector.tensor_tensor_scan(
                loc[:, sl], phw, phw, 0.0, op0=AL.add, op1=AL.bypass)
            nc.vector.scalar_tensor_tensor(
                junk2[:], loc[:, m * 128:m * 128 + 127], tau[:, m:m + 1],
                crw[:, 1:128], op0=AL.is_le, op1=AL.mult,
                accum_out=f1[:, m:m + 1])
        crw0 = fin.tile([128, NM], F32, tag="crw0", name="crw0")
        nc.vector.tensor_copy(crw0[:], wins[:, 128:NM * 256:256])

        # negG[m] = A_m + F_m - total_c  (so snr_bin = negG[m+1] - negG[m])
        negg = fin.tile([128, NM + 1], F32, tag="negg", name="negg")
        nc.vector.memset(negg[:, NM:NM + 1], 0.0)
        nc.vector.tensor_tensor(negg[:, 0:NM], acc_a[:], f1[:], op=AL.add)
        nc.vector.tensor_tensor(negg[:, 0:NM], negg[:, 0:NM], crw0[:], op=AL.add)
        nc.vector.tensor_scalar(negg[:, 0:NM], negg[:, 0:NM], tc_, None,
                                op0=AL.subtract)
        snr = fin.tile([128, NM], F32, tag="snr", name="snr")
        nc.vector.tensor_tensor(snr[:], negg[:, 1:NM + 1], negg[:, 0:NM],
                                op=AL.subtract)
        s16 = fin.tile([128, 1], F32, tag="s16", name="s16")
        nc.vector.tensor_scalar_mul(s16[:], tc_, 1.0 / NUM_BINS)
        ee = fin.tile([128, NM], F32, tag="ee", name="ee")
        nc.vector.tensor_scalar(ee[:], snr[:], s16[:], None, op0=AL.subtract)
        esq = fin.tile([128, NM], F32, tag="esq", name="esq")
        nc.vector.tensor_tensor(esq[:], ee[:], ee[:], op=AL.mult)
        ssum = fin.tile([128, 1], F32, tag="ssum", name="ssum")
        nc.vector.tensor_reduce(ssum[:], esq[:], op=AL.add, axis=AX.X)
        rth = fin.tile([128, 1], F32, tag="rth", name="rth")
        nc.vector.reciprocal(rth[:], th)
        chq = fin.tile([128, 1], F32, tag="chq", name="chq")
        nc.vector.tensor_tensor(chq[:], ssum[:], rth[:], op=AL.mult)
        nc.vector.tensor_scalar_mul(chq[:], chq[:],
                                    float(NUM_BINS) / (NUM_BINS - 1))
        nc.sync.dma_start(out[:], chq[:])

    nc.compile()
    return nc, _make_consts()


_CACHE = {}


def kernel(template: np.ndarray, strain: np.ndarray) -> np.ndarray:
    if "nc" not in _CACHE:
        _CACHE["nc"], _CACHE["consts"] = _build_program()
    nc, consts = _CACHE["nc"], _CACHE["consts"]

    t = np.ascontiguousarray(np.asarray(template, np.float32).reshape(1024, N))
    s = np.ascontiguousarray(np.asarray(strain, np.float32).reshape(1024, N))
    in_maps = []
    for c in range(NCORES):
        m = {"t_in": t[c * ROWS:(c + 1) * ROWS],
             "s_in": s[c * ROWS:(c + 1) * ROWS]}
        m.update(consts)
        in_maps.append(m)
    res = run_bass_kernel_spmd(nc, in_maps, list(range(NCORES)))
    outs = [res.results[c]["chisq_out"].reshape(ROWS) for c in range(NCORES)]
    full = np.concatenate(outs).astype(np.float32)
    return full.reshape(512, 2)


if __name__ == "__main__":
    rng = np.random.default_rng(0)
    tpl = rng.standard_normal((512, 2, N), dtype=np.float32)
    st = rng.standard_normal((512, 2, N), dtype=np.float32)
    print(kernel(tpl, st)[:3])
